# revision 4
# baseline (speedup 1.0000x reference)
"""AttnBlock3D (GroupNorm + single-head self-attention + proj + residual) on 8 trn2 cores.

Sharding: core i handles (batch b = i//4, query-block qb = i%4) of 1024 query
positions. Attention is permutation-equivariant over positions, so each core
receives its batch's x with the position axis rolled so that its query block
occupies columns 0:1024. Each core computes GroupNorm + full V for its batch
(4x replicated within a batch group) and attention/proj/residual for its own
1024 query positions. No collectives.

Algebraic restructures (exact up to fp rounding):
  * Q and K projections are never materialized. With Wqk = Wk^T Wq and
    bqk = Wk^T bq (host-computed),
      scores^T[nk, nq] = xn[:, nk] . (Wqk xn[:, :1024] + bqk)[:, nq]
                         + (per-nq constants, which cancel in softmax).
  * Softmax skips the max subtraction and the normalization is deferred past
    the output projection, so the rowsum -> reciprocal -> broadcast chain
    never gates a matmul.
  * The rowsum is a ones-vector matmul on the exp tiles (cross-partition
    reduction), broadcast back across partitions by a rank-1 f32 matmul.

fp8 DoubleRow: the B, V, scores and AV matmuls (+ rowsum) run as fp8e4
DoubleRow matmuls -- 2 fp8 MACs per PE cell per cycle, halving matmul count.
Both operands are [128, 2, free] planar slices (two adjacent 128-deep
contraction tiles). Scale management (exact, cancels in the deferred
normalization or host-folded):
  * Wqk and Wv are prescaled x16 on the host so their fp8 encodings sit in
    N(0,0.7) instead of the subnormal range; the exp scale absorbs Wqk's 16
    and V's eviction rescales by 1/16.
  * exp gets bias -ln4 so p stays under ~60 (fp8e4 max 240); the factor 1/4
    hits rowsum and AV identically and cancels in the normalization.
x arrives as bf16 (host-cast): halves the head DMA; residual/GN error ~0.4%
is far inside the 2e-2 gate. proj stays fp16 (o can reach ~6e2).

GroupNorm: per-channel mean/var via bn_stats/bn_aggr as x pieces land, then a
cross-partition group reduce and per-channel broadcast via tiny matmuls with
host-built selection matrices. PE warm-up matmuls paced by the x DMA keep the
HAM activity monitor at 2.4 GHz through the load phase.
"""

import math

import numpy as np
import ml_dtypes

import concourse.bass as bass
import concourse.tile as tile
from concourse import bacc, mybir
from concourse.bass import ds, ts
from concourse.bass_utils import run_bass_kernel_spmd

B, C, H, W, D = 2, 512, 16, 16, 16
N = H * W * D              # 4096 positions
NQ = N // 4                # 1024 query positions per core
T = C // 128               # 4 channel tiles
NKT = N // 128             # 32 key tiles
NKP = NKT // 2             # 16 key-tile pairs (DoubleRow granularity)
NQC = NQ // 512            # 2 query chunks of 512
NCH = N // 512             # 8 column chunks of 512
GROUPS = 32
GSIZE = C // GROUPS        # 16 channels per group
EPS = 1e-6
SCALE = float(C) ** -0.5
WS = 16.0                  # host prescale on Wqk / Wv (fp8 subnormal avoidance)
ESC = SCALE / WS           # exp scale (absorbs Wqk's x16)
EBIAS = -math.log(4.0)     # exp bias: p stays < ~60; cancels in normalization

F32 = mybir.dt.float32
F16 = mybir.dt.float16
BF16 = mybir.dt.bfloat16
F8 = mybir.dt.float8e4
DR = mybir.MatmulPerfMode.DoubleRow


def build_nc(reps: int = 1):
    nc = bacc.Bacc("TRN2", target_bir_lowering=False)

    env = {}
    env["x_d"] = nc.dram_tensor("x", [C, N], BF16, kind="ExternalInput")
    env["wqkT_d"] = nc.dram_tensor("wqkT", [C, C], F8, kind="ExternalInput")
    env["wvT_d"] = nc.dram_tensor("wvT", [C, C], F8, kind="ExternalInput")
    env["wpT_d"] = nc.dram_tensor("wpT", [C, C], F16, kind="ExternalInput")
    env["bqk_d"] = nc.dram_tensor("bqk", [128, T], F32, kind="ExternalInput")
    env["gnw_d"] = nc.dram_tensor("gnw", [128, T], F32, kind="ExternalInput")
    env["gnb_d"] = nc.dram_tensor("gnb", [128, T], F32, kind="ExternalInput")
    env["fb_d"] = nc.dram_tensor("fb", [128, T], F32, kind="ExternalInput")
    env["selred_d"] = nc.dram_tensor("selred", [128, T, GROUPS], F32, kind="ExternalInput")
    env["selbc_d"] = nc.dram_tensor("selbc", [GROUPS, C], F32, kind="ExternalInput")
    env["out_d"] = nc.dram_tensor("out", [C, NQ], F32, kind="ExternalOutput")

    with tile.TileContext(nc) as tc:
        import contextlib

        with contextlib.ExitStack() as ctx:
            env["const"] = ctx.enter_context(tc.tile_pool(name="const", bufs=1))
            env["big"] = ctx.enter_context(tc.tile_pool(name="big", bufs=1))
            env["mid"] = ctx.enter_context(tc.tile_pool(name="mid", bufs=1))
            env["stats"] = ctx.enter_context(tc.tile_pool(name="stats", bufs=2))
            env["small"] = ctx.enter_context(tc.tile_pool(name="small", bufs=2))
            env["ppool"] = ctx.enter_context(tc.tile_pool(name="ppool", bufs=2))
            env["ps_work"] = ctx.enter_context(tc.tile_pool(name="ps_work", bufs=2, space="PSUM"))
            env["ps_o"] = ctx.enter_context(tc.tile_pool(name="ps_o", bufs=4, space="PSUM"))
            env["ps_rs"] = ctx.enter_context(tc.tile_pool(name="ps_rs", bufs=1, space="PSUM"))
            env["ps_small"] = ctx.enter_context(tc.tile_pool(name="ps_small", bufs=1, space="PSUM"))

            const = env["const"]
            ones8p = const.tile([128, 2, 16], F8, tag="ones8p")
            nc.vector.memset(ones8p, 1.0)
            env["ones8p"] = ones8p
            ones32 = const.tile([1, 128], F32, tag="ones32")
            nc.vector.memset(ones32, 1.0)
            env["ones32"] = ones32
            epst = const.tile([GROUPS, 1], F32, tag="epst")
            nc.vector.memset(epst, EPS)
            env["epst"] = epst
            oneswu = const.tile([128, 1], BF16, tag="oneswu")
            nc.vector.memset(oneswu, 1.0)
            env["oneswu"] = oneswu
            ebias = const.tile([128, 1], F32, tag="ebias")
            nc.vector.memset(ebias, EBIAS)
            env["ebias"] = ebias

            for rep in range(reps):
                body(nc, tc, env, first=(rep == 0))

    nc.compile()
    return nc


def body(nc, tc, env, first=True):
    big, mid, stats, small, ppool = (env[k] for k in ("big", "mid", "stats", "small", "ppool"))
    ps_work, ps_o, ps_rs, ps_small = (env[k] for k in ("ps_work", "ps_o", "ps_rs", "ps_small"))
    x_d, out_d = env["x_d"], env["out_d"]
    const = env["const"]
    ones8p, ones32, epst, oneswu, ebias = (env[k] for k in ("ones8p", "ones32", "epst", "oneswu", "ebias"))
    AF = mybir.ActivationFunctionType

    # -------- load x (first, it gates everything) + GroupNorm stats --------
    # x arrives in bf16 (host-cast) in ~0.5MB pieces; bn_stats runs per
    # 512-chunk as pieces land. Dummy warm-up matmuls paced by the arriving
    # pieces keep the PE's HAM clock at 2.4 GHz through the load phase.
    x_sb = big.tile([128, T, N], BF16, tag="x")
    sts = []
    for t in range(T):
        st = stats.tile([128, NCH, 6], F32, tag=f"bnstats{t}", bufs=1, name=f"st{t}")
        sts.append(st)
    # PE warm-up: the HAM clock gate needs a ~3.4us DENSE busy window to
    # un-throttle; a dense bf16 dummy-matmul block paced by the first x piece
    # triggers it, and per-piece singles keep every later activity window
    # non-idle until the real stream begins.
    for t in range(T):
        eng = nc.sync if t % 2 == 0 else nc.scalar
        pieces = 1 if t < T - 1 else 4
        for piece in range(pieces):
            w = N // pieces
            eng.dma_start(out=x_sb[:, t, ds(piece * w, w)],
                          in_=x_d[ts(t, 128), ds(piece * w, w)])
            for s in range(piece * (NCH // pieces), (piece + 1) * (NCH // pieces)):
                nc.vector.bn_stats(out=sts[t][:, s, :],
                                   in_=x_sb[:, t, ds(s * 512, 512)])
            n_wu = (14 if t < T - 1 else 4) // pieces * pieces // pieces
            for wu in range(n_wu):
                wu_ps = ps_rs.tile([1, 256], F32, tag="psrs",
                                   name=f"wu{t}_{piece}_{wu}")
                nc.tensor.matmul(wu_ps, oneswu,
                                 x_sb[:, t, ds(piece * w + (wu % (w // 256)) * 256, 256)],
                                 start=True, stop=True)

    # -------- constants (after x in DMA priority; loaded once) --------
    if first:
        for nm, dt_ in (("wqkT", F8), ("wvT", F8), ("wpT", F16)):
            sb = const.tile([128, T, C], dt_, tag=nm, name=f"sb_{nm}")
            dr_ = env[f"{nm}_d"]
            for t in range(T):
                nc.sync.dma_start(out=sb[:, t, :], in_=dr_[ts(t, 128), :])
            env[nm] = sb
        for nm in ("bqk", "gnw", "gnb", "fb"):
            sb = const.tile([128, T], F32, tag=nm, name=f"sb_{nm}")
            nc.sync.dma_start(out=sb, in_=env[f"{nm}_d"][:, :])
            env[nm] = sb
        selred = const.tile([128, T, GROUPS], F32, tag="selred")
        nc.sync.dma_start(out=selred, in_=env["selred_d"][:, :, :])
        env["selred"] = selred
        selbc = const.tile([GROUPS, C], F32, tag="selbc")
        nc.sync.dma_start(out=selbc, in_=env["selbc_d"][:, :])
        env["selbc"] = selbc
    wqkT, wvT, wpT = env["wqkT"], env["wvT"], env["wpT"]
    bqk, gnw, gnb, fb = env["bqk"], env["gnw"], env["gnb"], env["fb"]
    selred, selbc = env["selred"], env["selbc"]

    # -------- finish GroupNorm statistics --------
    mvs = []
    for t in range(T):
        mv = stats.tile([128, 2], F32, tag=f"mv{t}", bufs=1, name=f"mv{t}")
        nc.vector.bn_aggr(out=mv, in_=sts[t])
        # mv := (mean, E[x^2]) ; E[x^2] = var + mean^2
        msq = stats.tile([128, 1], F32, tag="msq")
        nc.vector.tensor_mul(msq, mv[:, 0:1], mv[:, 0:1])
        nc.vector.tensor_add(mv[:, 1:2], mv[:, 1:2], msq)
        mvs.append(mv)

    psg = ps_small.tile([GROUPS, 2], F32, tag="pssmall")
    for t in range(T):
        nc.tensor.matmul(psg, selred[:, t, :], mvs[t], start=(t == 0), stop=(t == T - 1))

    # group scale/offset: rstd = 1/sqrt(var+eps), offset = -mean*rstd
    psgs = small.tile([GROUPS, 2], F32, tag="psgs", bufs=1)
    nc.vector.tensor_copy(psgs, psg)
    gsc = small.tile([GROUPS, 2], F32, tag="gsc", bufs=1)
    gtmp = small.tile([GROUPS, 2], F32, tag="gtmp", bufs=1)
    nc.vector.tensor_mul(gtmp[:, 0:1], psgs[:, 0:1], psgs[:, 0:1])      # mean^2
    nc.vector.tensor_sub(gtmp[:, 1:2], psgs[:, 1:2], gtmp[:, 0:1])      # var
    nc.scalar.activation(out=gsc[:, 0:1], in_=gtmp[:, 1:2], func=AF.Sqrt, bias=epst)
    nc.vector.reciprocal(gsc[:, 0:1], gsc[:, 0:1])                      # rstd
    nc.vector.tensor_mul(gsc[:, 1:2], psgs[:, 0:1], gsc[:, 0:1])       # mean*rstd
    nc.vector.tensor_scalar_mul(gsc[:, 1:2], gsc[:, 1:2], -1.0)        # offset

    # broadcast to per-channel scale/offset, fold gn weight/bias
    scof = small.tile([128, T, 2], F32, tag="scof", bufs=1)
    for t in range(T):
        psbc = ps_small.tile([128, 2], F32, tag="pssmall")
        nc.tensor.matmul(psbc, selbc[:, ts(t, 128)], gsc, start=True, stop=True)
        nc.vector.tensor_mul(scof[:, t, 0:1], psbc[:, 0:1], gnw[:, t:t + 1])
        nc.vector.tensor_mul(scof[:, t, 1:2], psbc[:, 1:2], gnw[:, t:t + 1])
        nc.vector.tensor_add(scof[:, t, 1:2], scof[:, t, 1:2], gnb[:, t:t + 1])

    # -------- apply GN -> xn (fp8e4), n-chunked so consumers pipeline --------
    xn = mid.tile([128, T, N], F8, tag="xn")
    for nch in range(NCH):
        for t in range(T):
            nc.vector.tensor_scalar(
                out=xn[:, t, ds(nch * 512, 512)], in0=x_sb[:, t, ds(nch * 512, 512)],
                scalar1=scof[:, t, 0:1], scalar2=scof[:, t, 1:2],
                op0=mybir.AluOpType.mult, op1=mybir.AluOpType.add,
            )

    # -------- B = Wqk xn_q + bqk  (fp8 DoubleRow over channel pairs) --------
    b_sb = mid.tile([128, T, NQ], F8, tag="b")
    for t_out in range(T):
        for nch in range(NQC):
            ps = ps_work.tile([128, 512], F32, tag="pswork")
            for g in range(T // 2):
                nc.tensor.matmul(ps, wqkT[:, 2 * g:2 * g + 2, ts(t_out, 128)],
                                 xn[:, 2 * g:2 * g + 2, ds(nch * 512, 512)],
                                 start=(g == 0), stop=(g == T // 2 - 1),
                                 perf_mode=DR)
            nc.scalar.activation(out=b_sb[:, t_out, ds(nch * 512, 512)], in_=ps,
                                 func=AF.Identity, bias=bqk[:, t_out:t_out + 1])

    # -------- V^T (fp8 DoubleRow; eviction rescales Wv's x16 away) --------
    vT = big.tile([128, NKT, C], F8, tag="vT")
    for nkt in range(NKT):
        ps = ps_work.tile([128, 512], F32, tag="pswork")
        for g in range(T // 2):
            nc.tensor.matmul(ps, xn[:, 2 * g:2 * g + 2, ts(nkt, 128)],
                             wvT[:, 2 * g:2 * g + 2, :],
                             start=(g == 0), stop=(g == T // 2 - 1),
                             perf_mode=DR)
        nc.scalar.activation(out=vT[:, nkt, :], in_=ps, func=AF.Identity,
                             scale=1.0 / WS, bias=0.0)

    # residual slice + folded bias, loaded late (off the head's DMA critical
    # path; only needed by the proj/residual stage)
    xq16 = mid.tile([128, T, NQ], BF16, tag="xq16")
    xq = mid.tile([128, T, NQ], F32, tag="xq")
    for t in range(T):
        nc.sync.dma_start(out=xq16[:, t, :], in_=x_d[ts(t, 128), 0:NQ])
        nc.vector.tensor_scalar_add(xq[:, t, :], xq16[:, t, :], fb[:, t:t + 1])

    # -------- attention + proj per query chunk --------
    # The PE-side epilogue of chunk ch (rowsum-broadcast matmul + proj) is
    # deferred into the middle of chunk ch+1's k-loop: by then the reciprocal
    # is long done, so the PE never stalls on the normalization chain, and no
    # >2us PE gap opens at the chunk boundary (which would trip the HAM
    # clock gate into its half-rate state).
    def pe_epilogue(ch):
        bc_ps = ps_small.tile([128, 512], F32, tag="pssmall", name=f"bcps{ch}")
        nc.tensor.matmul(bc_ps, ones32, env[f"rsinv{ch}"], start=True, stop=True)
        bc_sb = small.tile([128, 512], F32, tag="bc", name=f"bcsb{ch}")
        nc.vector.tensor_copy(bc_sb, bc_ps)
        for t_out in range(T):
            # chunk 0: head bank (keeps ps_work free for chunk 1's scores);
            # final chunk: rotate 3 slots (2x ps_work + head bank) so its four
            # accumulation groups don't serialize against the epilogue reads
            if ch == 0 or t_out == 0:
                ps = ps_small.tile([128, 512], F32, tag="pssmall", name=f"prps{ch}_{t_out}")
            else:
                ps = ps_work.tile([128, 512], F32, tag="pswork", name=f"prps{ch}_{t_out}")
            for tc_in in range(T):
                nc.tensor.matmul(ps, wpT[:, tc_in, ts(t_out, 128)],
                                 o_sb[:, tc_in, ds(ch * 512, 512)],
                                 start=(tc_in == 0), stop=(tc_in == T - 1))
            pn = small.tile([128, 512], F32, tag="pn", name=f"pn{ch}_{t_out}")
            nc.vector.tensor_mul(pn, ps, bc_sb)
            nc.vector.tensor_add(xq[:, t_out, ds(ch * 512, 512)],
                                 xq[:, t_out, ds(ch * 512, 512)], pn)
            nc.sync.dma_start(out=out_d[ts(t_out, 128), ds(ch * 512, 512)],
                              in_=xq[:, t_out, ds(ch * 512, 512)])

    o_sb = mid.tile([128, T, NQ], F16, tag="o")
    for ch in range(NQC):
        o_ps = [ps_o.tile([128, 512], F32, tag="pso", name=f"ops{ch}_{i}")
                for i in range(T)]
        rs_ps = ps_rs.tile([1, 512], F32, tag="psrs")
        p8 = ppool.tile([128, NKT, 512], F8, tag="p")
        # Software-pipelined by one pair: pair j+1's score matmuls are
        # emitted between pair j's scores and pair j's AV so the PE has work
        # while the exp (ScalarE) for pair j is still in flight.
        def emit_av(j):
            # rowsum first: its 2-column LDWEIGHTS is nearly free and fills
            # the pipeline while the exp->AV semaphore settles
            nc.tensor.matmul(rs_ps, ones8p[:, :, 0:1], p8[:, 2 * j:2 * j + 2, :],
                             start=(j == 0), stop=(j == NKP - 1), perf_mode=DR)
            for tc_in in range(T):
                nc.tensor.matmul(o_ps[tc_in], vT[:, 2 * j:2 * j + 2, ts(tc_in, 128)],
                                 p8[:, 2 * j:2 * j + 2, :],
                                 start=(j == 0), stop=(j == NKP - 1), perf_mode=DR)

        prev = None
        for j in range(NKP):
            for h in range(2):
                nkt = 2 * j + h
                s_ps = ps_work.tile([128, 512], F32, tag="pswork")
                for g in range(T // 2):
                    nc.tensor.matmul(s_ps, xn[:, 2 * g:2 * g + 2, ts(nkt, 128)],
                                     b_sb[:, 2 * g:2 * g + 2, ds(ch * 512, 512)],
                                     start=(g == 0), stop=(g == T // 2 - 1),
                                     perf_mode=DR)
                nc.scalar.activation(out=p8[:, nkt, :], in_=s_ps, func=AF.Exp,
                                     scale=ESC, bias=ebias)
            if prev is not None:
                emit_av(prev)
            prev = j
            if ch > 0 and j == 3:
                pe_epilogue(ch - 1)
        emit_av(prev)

        # rowsum reciprocal (DVE) + unnormalized-o eviction (ScalarE) happen
        # immediately — the eviction frees the o accumulators for the next
        # chunk's AV matmuls. |o| < ~7e2, safely fp16.
        rsinv = small.tile([1, 512], F32, tag="rsinv", name=f"rsinv{ch}")
        nc.vector.reciprocal(rsinv, rs_ps)
        env[f"rsinv{ch}"] = rsinv
        # evictions split across ScalarE and VectorE to halve the latency
        # before the next chunk's AV matmuls can claim the o accumulators
        for tc_in in range(T):
            if tc_in % 2 == 0:
                nc.scalar.activation(out=o_sb[:, tc_in, ds(ch * 512, 512)],
                                     in_=o_ps[tc_in], func=AF.Identity, bias=0.0)
            else:
                nc.vector.tensor_copy(o_sb[:, tc_in, ds(ch * 512, 512)],
                                      o_ps[tc_in])

    pe_epilogue(NQC - 1)


_NC_CACHE = {}


def _get_nc(reps: int = 1):
    if reps not in _NC_CACHE:
        _NC_CACHE[reps] = build_nc(reps)
    return _NC_CACHE[reps]


def make_in_maps(x, gn_weight, gn_bias, qkv_weight, qkv_bias, proj_weight, proj_bias):
    x = np.asarray(x, np.float32)
    qkv_weight = np.asarray(qkv_weight, np.float32)
    proj_weight = np.asarray(proj_weight, np.float32)
    qkv_bias = np.asarray(qkv_bias, np.float32)
    proj_bias = np.asarray(proj_bias, np.float32)
    gn_weight = np.asarray(gn_weight, np.float32)
    gn_bias = np.asarray(gn_bias, np.float32)

    Wq, Wk, Wv = qkv_weight[0:C], qkv_weight[C:2 * C], qkv_weight[2 * C:3 * C]
    wqkT = np.ascontiguousarray((WS * (Wq.T @ Wk)).astype(ml_dtypes.float8_e4m3))
    wvT = np.ascontiguousarray((WS * Wv.T).astype(ml_dtypes.float8_e4m3))
    wpT = np.ascontiguousarray(proj_weight.T.astype(np.float16))

    def cols(v):  # [C] -> [128, T]
        return np.ascontiguousarray(v.reshape(T, 128).T.astype(np.float32))

    bqkv = WS * (Wk.T @ qkv_bias[0:C])
    fbv = proj_weight @ qkv_bias[2 * C:3 * C] + proj_bias

    p_idx = np.arange(128)
    selred = np.zeros((128, T, GROUPS), np.float32)
    selbc = np.zeros((GROUPS, C), np.float32)
    for t in range(T):
        g = t * (128 // GSIZE) + p_idx // GSIZE
        selred[p_idx, t, g] = 1.0 / GSIZE
        selbc[g, t * 128 + p_idx] = 1.0

    shared = {
        "wqkT": wqkT, "wvT": wvT, "wpT": wpT,
        "bqk": cols(bqkv),
        "gnw": cols(gn_weight), "gnb": cols(gn_bias), "fb": cols(fbv),
        "selred": selred, "selbc": selbc,
    }
    in_maps = []
    for core in range(8):
        b, qb = core // 4, core % 4
        xb = x[b].reshape(C, N)
        xr = np.ascontiguousarray(
            np.roll(xb, -qb * NQ, axis=1).astype(ml_dtypes.bfloat16))
        m = dict(shared)
        m["x"] = xr
        in_maps.append(m)
    return in_maps


def kernel(x, gn_weight, gn_bias, qkv_weight, qkv_bias, proj_weight, proj_bias):
    nc = _get_nc(1)
    in_maps = make_in_maps(x, gn_weight, gn_bias, qkv_weight, qkv_bias,
                           proj_weight, proj_bias)
    res = run_bass_kernel_spmd(nc, in_maps, core_ids=list(range(8)))
    out = np.empty((B, C, N), np.float32)
    for core in range(8):
        b, qb = core // 4, core % 4
        out[b][:, qb * NQ:(qb + 1) * NQ] = res.results[core]["out"]
    return out.reshape(B, C, H, W, D)


# revision 15
# speedup vs baseline: 1.0196x; 1.0196x over previous
"""AttnBlock3D (GroupNorm + single-head self-attention + proj + residual) on 8 trn2 cores.

Sharding: core i handles (batch b = i//4, query-block qb = i%4) of 1024 query
positions. Attention is permutation-equivariant over positions, so each core
receives its batch's x with the position axis rolled so that its query block
occupies columns 0:1024. Each core computes GroupNorm + full V for its batch
(4x replicated within a batch group) and attention/proj/residual for its own
1024 query positions. No collectives.

Algebraic restructures (exact up to fp rounding):
  * Q and K projections are never materialized. With Wqk = Wk^T Wq and
    bqk = Wk^T bq (host-computed),
      scores^T[nk, nq] = xn[:, nk] . (Wqk xn[:, :1024] + bqk)[:, nq]
                         + (per-nq constants, which cancel in softmax).
  * Softmax skips the max subtraction and the normalization is deferred past
    the output projection (folded into the o eviction), so the
    rowsum -> reciprocal chain never gates a matmul.
  * The rowsum matmul uses an all-0.0625 [128,2,128] stationary, so the
    per-query sum lands broadcast across all 128 PSUM partitions -- the
    reciprocal then runs as a full-width [128,512] DVE op (a [1,512]
    single-partition reciprocal costs 4us; this costs 0.7us).

fp8 DoubleRow: ALL five matmul families (B, V, scores, AV+rowsum, proj) run
as fp8e4 DoubleRow -- 2 fp8 MACs per PE cell per cycle. Operands are
[128, 2, free] planar slices. Scale management (exact, cancels in the
deferred normalization or host-folded):
  * Wqk, Wv, Wp are prescaled x16 on the host so their fp8 encodings sit at
    N(0,0.7) instead of the subnormal range. The exp scale absorbs Wqk's 16;
    V's eviction rescales by 1/16; proj's 16 is folded into the final
    residual-add's 1/256 scalar.
  * exp gets bias -ln16 so p stays < ~15 (fp8e4 max 240) and the
    unnormalized o stays < ~160 (fp8-safe after normalization x256).
  * o is normalized AT eviction (o8 = o_psum * (256/rowsum), ~N(0,0.4)), so
    the epilogue is proj -> one fused (ps/256 + xq) scalar_tensor_tensor.
x arrives as bf16 (host-cast): halves the head DMA; residual/GN error ~0.4%
is far inside the 2e-2 gate.

Engine assignment: ScalarE runs ONLY Sqrt (GN, once) and Exp (64 tiles) --
every PSUM eviction runs on DVE and the GN-apply runs on GpSimd+DVE, so the
1.5us ACT_TABLE_LOAD fires exactly twice, both during the DMA-bound head
(the Exp table via a dummy activation right after the GN Sqrt).

Head: x lands in 8 half-tile (2048-col) pieces alternating across both DMA
queues; bn_stats runs per 2048 piece (fixed cost amortization), each tile's
bn_aggr + group-reduce matmul emitted right after its pieces so the DVE
drains the GroupNorm chain while later pieces are still in flight. PE
warm-up matmuls paced by the pieces and the per-tile group matmuls keep the
HAM activity monitor at 2.4 GHz through the whole load phase.
"""

import math

import numpy as np
import ml_dtypes

import concourse.bass as bass
import concourse.tile as tile
from concourse import bacc, mybir
from concourse.bass import ds, ts
from concourse.bass_utils import run_bass_kernel_spmd

B, C, H, W, D = 2, 512, 16, 16, 16
N = H * W * D              # 4096 positions
NQ = N // 4                # 1024 query positions per core
T = C // 128               # 4 channel tiles
NKT = N // 128             # 32 key tiles
NKP = NKT // 2             # 16 key-tile pairs (DoubleRow granularity)
NQC = NQ // 512            # 2 query chunks of 512
GROUPS = 32
GSIZE = C // GROUPS        # 16 channels per group
EPS = 1e-6
SCALE = float(C) ** -0.5
WS = 16.0                  # host prescale on Wqk / Wv / Wp (fp8 subnormal avoidance)
ESC = SCALE / WS           # exp scale (absorbs Wqk's x16)
EBIAS = -math.log(16.0)    # exp bias: p < ~15, o_psum < ~160; cancels via rowsum
RSONE = 1.0 / 16.0         # rowsum stationary value -> rsinv = 256/rowsum
PRSC = 1.0 / 256.0         # epilogue scalar: (16 Wp)(16 att) -> /256

F32 = mybir.dt.float32
F16 = mybir.dt.float16
BF16 = mybir.dt.bfloat16
F8 = mybir.dt.float8e4
DR = mybir.MatmulPerfMode.DoubleRow


def build_nc(reps: int = 1):
    nc = bacc.Bacc("TRN2", target_bir_lowering=False)

    env = {}
    env["x_d"] = nc.dram_tensor("x", [C, N], BF16, kind="ExternalInput")
    env["wqkT_d"] = nc.dram_tensor("wqkT", [C, C], F8, kind="ExternalInput")
    env["wvT_d"] = nc.dram_tensor("wvT", [C, C], F8, kind="ExternalInput")
    env["wpT_d"] = nc.dram_tensor("wpT", [C, C], F8, kind="ExternalInput")
    env["bqk_d"] = nc.dram_tensor("bqk", [128, T], F32, kind="ExternalInput")
    env["gnw_d"] = nc.dram_tensor("gnw", [128, T], F32, kind="ExternalInput")
    env["gnb_d"] = nc.dram_tensor("gnb", [128, T], F32, kind="ExternalInput")
    env["fb_d"] = nc.dram_tensor("fb", [128, T], F32, kind="ExternalInput")
    env["selred_d"] = nc.dram_tensor("selred", [128, T, GROUPS], F32, kind="ExternalInput")
    env["selbc_d"] = nc.dram_tensor("selbc", [GROUPS, C], F32, kind="ExternalInput")
    env["out_d"] = nc.dram_tensor("out", [C, NQ], F32, kind="ExternalOutput")

    with tile.TileContext(nc) as tc:
        import contextlib

        with contextlib.ExitStack() as ctx:
            env["const"] = ctx.enter_context(tc.tile_pool(name="const", bufs=1))
            env["big"] = ctx.enter_context(tc.tile_pool(name="big", bufs=1))
            env["mid"] = ctx.enter_context(tc.tile_pool(name="mid", bufs=1))
            env["stats"] = ctx.enter_context(tc.tile_pool(name="stats", bufs=2))
            env["small"] = ctx.enter_context(tc.tile_pool(name="small", bufs=2))
            env["ppool"] = ctx.enter_context(tc.tile_pool(name="ppool", bufs=2))
            env["ps_work"] = ctx.enter_context(tc.tile_pool(name="ps_work", bufs=2, space="PSUM"))
            env["ps_o"] = ctx.enter_context(tc.tile_pool(name="ps_o", bufs=4, space="PSUM"))
            env["ps_rs"] = ctx.enter_context(tc.tile_pool(name="ps_rs", bufs=1, space="PSUM"))
            env["ps_small"] = ctx.enter_context(tc.tile_pool(name="ps_small", bufs=1, space="PSUM"))

            const = env["const"]
            ones8b = const.tile([128, 2, 128], F8, tag="ones8b")
            nc.vector.memset(ones8b, RSONE)
            env["ones8b"] = ones8b
            epst = const.tile([GROUPS, 1], F32, tag="epst")
            nc.vector.memset(epst, EPS)
            env["epst"] = epst
            oneswu = const.tile([128, 1], BF16, tag="oneswu")
            nc.vector.memset(oneswu, 1.0)
            env["oneswu"] = oneswu
            ebias = const.tile([128, 1], F32, tag="ebias")
            nc.vector.memset(ebias, EBIAS)
            env["ebias"] = ebias
            dummy = const.tile([128, 1], F32, tag="dummy")
            env["dummy"] = dummy
            ones32c = const.tile([128, 1], F32, tag="ones32c")
            nc.vector.memset(ones32c, 1.0)
            env["ones32c"] = ones32c

            for rep in range(reps):
                body(nc, tc, env, first=(rep == 0))

    nc.compile()
    return nc


def body(nc, tc, env, first=True):
    big, mid, stats, small, ppool = (env[k] for k in ("big", "mid", "stats", "small", "ppool"))
    ps_work, ps_o, ps_rs, ps_small = (env[k] for k in ("ps_work", "ps_o", "ps_rs", "ps_small"))
    x_d, out_d = env["x_d"], env["out_d"]
    const = env["const"]
    ones8b, epst, oneswu, ebias, dummy, ones32c = (
        env[k] for k in ("ones8b", "epst", "oneswu", "ebias", "dummy", "ones32c"))
    AF = mybir.ActivationFunctionType
    wu_count = [0]

    def warmups(n, t_avail, lo=0, width=N):
        # dummy bf16 matmuls on already-resident x pieces; placed in the PE
        # queue to fill would-be idle windows so the HAM clock gate never
        # re-arms. Reads only x_sb[:, t_avail, lo:lo+width] (landed data).
        for _ in range(n):
            i = wu_count[0]
            wu_count[0] += 1
            wu_ps = ps_rs.tile([1, 256], F32, tag="psrs", name=f"wu{i}")
            nc.tensor.matmul(wu_ps, oneswu,
                             x_sb[:, t_avail, ds(lo + (i % (width // 256)) * 256, 256)],
                             start=True, stop=True)

    def wu_paced(src):
        # tiny f32 matmul reading a just-produced DVE result: lands in the PE
        # queue right when the DVE finishes it, so sparse PE activity tracks
        # DVE progress through the stats phase (maintains the HAM un-throttle)
        i = wu_count[0]
        wu_count[0] += 1
        wu_ps = ps_rs.tile([1, src.shape[-1]], F32, tag="psrs", name=f"wp{i}")
        nc.tensor.matmul(wu_ps, ones32c[:src.shape[0], :], src, start=True, stop=True)

    # -------- load x + GroupNorm stats, pipelined per 2048-piece --------
    if first:
        def load_small_consts():
            # tiny (needed by the GN chain mid-head; ~0.1us of queue time)
            for nm in ("bqk", "gnw", "gnb", "fb"):
                sb = const.tile([128, T], F32, tag=nm, name=f"sb_{nm}")
                nc.scalar.dma_start(out=sb, in_=env[f"{nm}_d"][:, :])
                env[nm] = sb
            selred = const.tile([128, T, GROUPS], F32, tag="selred")
            nc.sync.dma_start(out=selred, in_=env["selred_d"][:, :, :])
            env["selred"] = selred
            selbc = const.tile([GROUPS, C], F32, tag="selbc")
            nc.sync.dma_start(out=selbc, in_=env["selbc_d"][:, :])
            env["selbc"] = selbc

        def load_weights():
            # 0.75MB: emitted after the last x piece so they never delay x
            for nm in ("wqkT", "wvT", "wpT"):
                sb = const.tile([128, T, C], F8, tag=nm, name=f"sb_{nm}")
                dr_ = env[f"{nm}_d"]
                for t in range(T):
                    eng = nc.scalar if t % 2 == 0 else nc.sync
                    eng.dma_start(out=sb[:, t, :], in_=dr_[ts(t, 128), :])
                env[nm] = sb

    x_sb = big.tile([128, T, N], BF16, tag="x")
    sts = []
    for t in range(T):
        st = stats.tile([128, 8, 6], F32, tag=f"bnstats{t}", bufs=1, name=f"st{t}")
        sts.append(st)

    psg = ps_small.tile([GROUPS, 2], F32, tag="pssmall")
    for t in range(T):
        for h in range(2):
            eng = nc.sync if h == 0 else nc.scalar
            eng.dma_start(out=x_sb[:, t, ds(h * 2048, 2048)],
                          in_=x_d[ts(t, 128), ds(h * 2048, 2048)])
            warmups(16 if (t, h) == (0, 0) else 2, t, lo=h * 2048, width=2048)
            for s in range(4):
                nc.vector.bn_stats(out=sts[t][:, 4 * h + s, :],
                                   in_=x_sb[:, t, ds(h * 2048 + s * 512, 512)])
                wu_paced(sts[t][:, 4 * h + s, :])
        if t == 0 and first:
            load_small_consts()
        # tile t's aggregation, emitted immediately so the DVE drains it
        # while later pieces are still in flight
        mv = stats.tile([128, 2], F32, tag=f"mv{t}", bufs=1, name=f"mv{t}")
        nc.vector.bn_aggr(out=mv, in_=sts[t])
        # mv := (mean, E[x^2]) ; E[x^2] = var + mean^2
        msq = stats.tile([128, 1], F32, tag="msq")
        nc.vector.tensor_mul(msq, mv[:, 0:1], mv[:, 0:1])
        nc.vector.tensor_add(mv[:, 1:2], mv[:, 1:2], msq)
        wu_paced(mv)
        nc.tensor.matmul(psg, env["selred"][:, t, :], mv,
                         start=(t == 0), stop=(t == T - 1))
    if first:
        load_weights()
    warmups(6, T - 1)

    wqkT, wvT, wpT = env["wqkT"], env["wvT"], env["wpT"]
    bqk, gnw, gnb, fb = env["bqk"], env["gnw"], env["gnb"], env["fb"]
    selbc = env["selbc"]

    # group scale/offset: rstd = 1/sqrt(var+eps), offset = -mean*rstd
    psgs = small.tile([GROUPS, 2], F32, tag="psgs", bufs=1)
    nc.vector.tensor_copy(psgs, psg)
    wu_paced(psgs)
    gsc = small.tile([GROUPS, 2], F32, tag="gsc", bufs=1)
    gtmp = small.tile([GROUPS, 2], F32, tag="gtmp", bufs=1)
    nc.vector.tensor_mul(gtmp[:, 0:1], psgs[:, 0:1], psgs[:, 0:1])      # mean^2
    nc.vector.tensor_sub(gtmp[:, 1:2], psgs[:, 1:2], gtmp[:, 0:1])      # var
    wu_paced(gtmp)
    nc.scalar.activation(out=gsc[:, 0:1], in_=gtmp[:, 1:2], func=AF.Sqrt, bias=epst)
    # preload the Exp activation table NOW (ScalarE idle; its next real use
    # is the first scores exp, which must not eat the 1.5us table load)
    nc.scalar.activation(out=dummy, in_=ebias, func=AF.Exp, bias=0.0)
    nc.vector.reciprocal(gsc[:, 0:1], gsc[:, 0:1])                      # rstd
    nc.vector.tensor_mul(gsc[:, 1:2], psgs[:, 0:1], gsc[:, 0:1])       # mean*rstd
    nc.vector.tensor_scalar_mul(gsc[:, 1:2], gsc[:, 1:2], -1.0)        # offset
    wu_paced(gsc)

    # broadcast to per-channel scale/offset, fold gn weight/bias; then apply
    # GN -> xn (fp8e4) on the DVE in 2048 pieces, query-half first so B and
    # the early V tiles unblock as soon as possible
    scof = small.tile([128, T, 2], F32, tag="scof", bufs=1)
    xn = mid.tile([128, T, N], F8, tag="xn")
    for t in range(T):
        psbc = ps_small.tile([128, 2], F32, tag="pssmall", name=f"psbc{t}")
        nc.tensor.matmul(psbc, selbc[:, ts(t, 128)], gsc, start=True, stop=True)
        nc.vector.tensor_mul(scof[:, t, 0:1], psbc[:, 0:1], gnw[:, t:t + 1])
        nc.vector.tensor_mul(scof[:, t, 1:2], psbc[:, 1:2], gnw[:, t:t + 1])
        nc.vector.tensor_add(scof[:, t, 1:2], scof[:, t, 1:2], gnb[:, t:t + 1])
    for h in range(2):
        for t in range(T):
            nc.vector.tensor_scalar(
                out=xn[:, t, ds(h * 2048, 2048)], in0=x_sb[:, t, ds(h * 2048, 2048)],
                scalar1=scof[:, t, 0:1], scalar2=scof[:, t, 1:2],
                op0=mybir.AluOpType.mult, op1=mybir.AluOpType.add,
            )
    warmups(4, T - 1)

    # -------- B = Wqk xn_q + bqk  (fp8 DoubleRow over channel pairs) --------
    b_sb = mid.tile([128, T, NQ], F8, tag="b")
    for t_out in range(T):
        for nch in range(NQC):
            ps = ps_work.tile([128, 512], F32, tag="pswork")
            for g in range(T // 2):
                nc.tensor.matmul(ps, wqkT[:, 2 * g:2 * g + 2, ts(t_out, 128)],
                                 xn[:, 2 * g:2 * g + 2, ds(nch * 512, 512)],
                                 start=(g == 0), stop=(g == T // 2 - 1),
                                 perf_mode=DR)
            nc.vector.tensor_scalar_add(b_sb[:, t_out, ds(nch * 512, 512)], ps,
                                        bqk[:, t_out:t_out + 1])

    # -------- V^T (fp8 DoubleRow; DVE eviction rescales Wv's x16 away) -----
    vT = big.tile([128, NKT, C], F8, tag="vT")
    for nkt in range(NKT):
        ps = ps_work.tile([128, 512], F32, tag="pswork")
        for g in range(T // 2):
            nc.tensor.matmul(ps, xn[:, 2 * g:2 * g + 2, ts(nkt, 128)],
                             wvT[:, 2 * g:2 * g + 2, :],
                             start=(g == 0), stop=(g == T // 2 - 1),
                             perf_mode=DR)
        nc.vector.tensor_scalar_mul(vT[:, nkt, :], ps, 1.0 / WS)

    # residual slice + folded bias, loaded late (off the head's DMA critical
    # path; only needed by the proj/residual stage)
    xq16 = mid.tile([128, T, NQ], BF16, tag="xq16")
    xq = mid.tile([128, T, NQ], F32, tag="xq")
    for t in range(T):
        nc.sync.dma_start(out=xq16[:, t, :], in_=x_d[ts(t, 128), 0:NQ])
        nc.vector.tensor_scalar_add(xq[:, t, :], xq16[:, t, :], fb[:, t:t + 1])

    # -------- attention + proj per query chunk --------
    # o is normalized at eviction; the PE epilogue (proj + fused residual
    # add) of chunk ch is deferred into chunk ch+1's k-loop.
    def pe_epilogue(ch):
        for t_out in range(T):
            if ch == 0 or t_out == 0:
                ps = ps_small.tile([128, 512], F32, tag="pssmall", name=f"prps{ch}_{t_out}")
            else:
                ps = ps_work.tile([128, 512], F32, tag="pswork", name=f"prps{ch}_{t_out}")
            for g in range(T // 2):
                nc.tensor.matmul(ps, wpT[:, 2 * g:2 * g + 2, ts(t_out, 128)],
                                 o_sb[:, 2 * g:2 * g + 2, ds(ch * 512, 512)],
                                 start=(g == 0), stop=(g == T // 2 - 1),
                                 perf_mode=DR)
            nc.vector.scalar_tensor_tensor(
                out=xq[:, t_out, ds(ch * 512, 512)], in0=ps, scalar=PRSC,
                in1=xq[:, t_out, ds(ch * 512, 512)],
                op0=mybir.AluOpType.mult, op1=mybir.AluOpType.add)
            nc.sync.dma_start(out=out_d[ts(t_out, 128), ds(ch * 512, 512)],
                              in_=xq[:, t_out, ds(ch * 512, 512)])

    o_sb = mid.tile([128, T, NQ], F8, tag="o")
    for ch in range(NQC):
        o_ps = [ps_o.tile([128, 512], F32, tag="pso", name=f"ops{ch}_{i}")
                for i in range(T)]
        rs_ps = ps_rs.tile([128, 512], F32, tag="psrs")
        p8 = ppool.tile([128, NKT, 512], F8, tag="p")
        # Software-pipelined by one pair: pair j+1's score matmuls are
        # emitted between pair j's scores and pair j's AV so the PE has work
        # while the exp (ScalarE) for pair j is still in flight.
        def emit_av(j):
            nc.tensor.matmul(rs_ps, ones8b, p8[:, 2 * j:2 * j + 2, :],
                             start=(j == 0), stop=(j == NKP - 1), perf_mode=DR)
            for tc_in in range(T):
                nc.tensor.matmul(o_ps[tc_in], vT[:, 2 * j:2 * j + 2, ts(tc_in, 128)],
                                 p8[:, 2 * j:2 * j + 2, :],
                                 start=(j == 0), stop=(j == NKP - 1), perf_mode=DR)

        prev = None
        for j in range(NKP):
            for h in range(2):
                nkt = 2 * j + h
                s_ps = ps_work.tile([128, 512], F32, tag="pswork")
                for g in range(T // 2):
                    nc.tensor.matmul(s_ps, xn[:, 2 * g:2 * g + 2, ts(nkt, 128)],
                                     b_sb[:, 2 * g:2 * g + 2, ds(ch * 512, 512)],
                                     start=(g == 0), stop=(g == T // 2 - 1),
                                     perf_mode=DR)
                nc.scalar.activation(out=p8[:, nkt, :], in_=s_ps, func=AF.Exp,
                                     scale=ESC, bias=ebias)
            if prev is not None:
                emit_av(prev)
            prev = j
            if ch > 0 and j == 3:
                pe_epilogue(ch - 1)
        emit_av(prev)

        # normalize-at-eviction: rsinv = 256/rowsum (full-width reciprocal),
        # o8 = o_psum * rsinv ~ N(0, 0.4); frees the o accumulators for the
        # next chunk's AV matmuls.
        rsinv = small.tile([128, 512], F32, tag="rsinv", name=f"rsinv{ch}")
        nc.vector.reciprocal(rsinv, rs_ps)
        for tc_in in range(T):
            nc.vector.tensor_mul(o_sb[:, tc_in, ds(ch * 512, 512)],
                                 o_ps[tc_in], rsinv)

    pe_epilogue(NQC - 1)


_NC_CACHE = {}


def _get_nc(reps: int = 1):
    if reps not in _NC_CACHE:
        _NC_CACHE[reps] = build_nc(reps)
    return _NC_CACHE[reps]


def make_in_maps(x, gn_weight, gn_bias, qkv_weight, qkv_bias, proj_weight, proj_bias):
    x = np.asarray(x, np.float32)
    qkv_weight = np.asarray(qkv_weight, np.float32)
    proj_weight = np.asarray(proj_weight, np.float32)
    qkv_bias = np.asarray(qkv_bias, np.float32)
    proj_bias = np.asarray(proj_bias, np.float32)
    gn_weight = np.asarray(gn_weight, np.float32)
    gn_bias = np.asarray(gn_bias, np.float32)

    Wq, Wk, Wv = qkv_weight[0:C], qkv_weight[C:2 * C], qkv_weight[2 * C:3 * C]
    wqkT = np.ascontiguousarray((WS * (Wq.T @ Wk)).astype(ml_dtypes.float8_e4m3))
    wvT = np.ascontiguousarray((WS * Wv.T).astype(ml_dtypes.float8_e4m3))
    wpT = np.ascontiguousarray((WS * proj_weight.T).astype(ml_dtypes.float8_e4m3))

    def cols(v):  # [C] -> [128, T]
        return np.ascontiguousarray(v.reshape(T, 128).T.astype(np.float32))

    bqkv = WS * (Wk.T @ qkv_bias[0:C])
    fbv = proj_weight @ qkv_bias[2 * C:3 * C] + proj_bias

    p_idx = np.arange(128)
    selred = np.zeros((128, T, GROUPS), np.float32)
    selbc = np.zeros((GROUPS, C), np.float32)
    for t in range(T):
        g = t * (128 // GSIZE) + p_idx // GSIZE
        selred[p_idx, t, g] = 1.0 / GSIZE
        selbc[g, t * 128 + p_idx] = 1.0

    shared = {
        "wqkT": wqkT, "wvT": wvT, "wpT": wpT,
        "bqk": cols(bqkv),
        "gnw": cols(gn_weight), "gnb": cols(gn_bias), "fb": cols(fbv),
        "selred": selred, "selbc": selbc,
    }
    in_maps = []
    for core in range(8):
        b, qb = core // 4, core % 4
        xb = x[b].reshape(C, N)
        xr = np.ascontiguousarray(
            np.roll(xb, -qb * NQ, axis=1).astype(ml_dtypes.bfloat16))
        m = dict(shared)
        m["x"] = xr
        in_maps.append(m)
    return in_maps


def kernel(x, gn_weight, gn_bias, qkv_weight, qkv_bias, proj_weight, proj_bias):
    nc = _get_nc(1)
    in_maps = make_in_maps(x, gn_weight, gn_bias, qkv_weight, qkv_bias,
                           proj_weight, proj_bias)
    res = run_bass_kernel_spmd(nc, in_maps, core_ids=list(range(8)))
    out = np.empty((B, C, N), np.float32)
    for core in range(8):
        b, qb = core // 4, core % 4
        out[b][:, qb * NQ:(qb + 1) * NQ] = res.results[core]["out"]
    return out.reshape(B, C, H, W, D)
